# revision 67
# baseline (speedup 1.0000x reference)
"""Trainium2 Bass kernel for nn_CondIndepenLoss (transposed / PE-reduce design).

Computes, for B=65536 rows sharded 8192/core over 8 NeuronCores:
    jp   = softmax(joint_probs[:, :64])                      [B, 64]
    lp[b,c] = sum_d log(softmax(pred_probs[d, b])[valid_cp[c, d]])
    w[b] = exp(-0.5*(|Z_b|^2 + |X_b - Xhat_b|^2))
    vals[b] = jp[b,y] * w[b] * (log(jp[b,y]) - lp[b,y]),  y = Y_valid[b]
    loss = |sum_b vals[b] * (y<64)| / count(y<64)

Key identities used: log(softmax(v)[i]) = v[i] - log(sum exp(v)); the +eps
inside the reference's logs is ~1e4x below the smallest softmax value here,
so it is dropped.  Selected logits (joint[b,y], pred[d,b,valid_cp[y,d]]) are
gathered on the host (pure index preprocessing); the device computes every
reduction/exp/log/product.

Hardware structure (per core, 8192 rows, all big streams bf16):
  - host supplies a TRANSPOSED stream TXZ=[X.T; Xhat.T; Z.T] [1152, 8192]
    and JPP=[joint.T; pred.T] [94, 8192]; feature dims sit on SBUF
    partitions, rows on the free axis
  - per 512-row iteration: VectorE subtracts x-xhat (bf16 2x mode) and
    squares 2 of 5 chunks; ScalarE squares the other 3 and exponentiates
    JPP; the idle TensorEngine then reduces over the partition axis:
    ldweights(squared chunk [128,128-rows]) x ones -> per-row |dx|^2+|z|^2,
    and ldweights(exp'd JPP [94,128-rows]) x 0/1 segment matrix ->
    (sum_c e^joint, sum_k e^pred_d for d=0..2), all landing in PSUM in
    [row%128 partition, row//128 column] layout
  - final pointwise math runs once over [128, 64] column buffers, a PE
    matmul against ones reduces across partitions, [1,2]=(sum,count) -> HBM
  - host combines the 8 per-core partials: loss = |sum|/count
"""

import os
import sys

import numpy as np

for _p in ("/opt/trn_rl_repo",):
    if os.path.isdir(_p) and _p not in sys.path:
        sys.path.insert(0, _p)

from contextlib import ExitStack

import ml_dtypes

from concourse import bacc, bass, mybir, tile
from concourse.bass_utils import run_bass_kernel_spmd

BF16NP = ml_dtypes.bfloat16
F8NP = ml_dtypes.float8_e4m3fn

M = 8                     # cores
B = 65536
BL = B // M               # 8192 rows per core
P = 128                   # SBUF partitions
XD, ZD, C, D, K = 512, 128, 64, 3, 10
NCH = 9                   # 1152/128 x|xhat|z chunks in the record stream
JH = C + D * K            # 94 live partitions in the joint|pred chunk
NREC = 10                 # chunks per record (x 4, xhat 4, z 1, jpp 1)
NI = 8                    # iterations per core
RI = BL // NI             # 1024 rows per iteration (16KB DMA descriptors)
GPI = RI // P             # 8 PE groups (128 rows) per iteration
NG = BL // P              # 64 groups / column slots per core
NGH = NG // 2             # groups per combine half
F32 = mybir.dt.float32
BF16 = mybir.dt.bfloat16
F8 = mybir.dt.float8e4

_NC_CACHE = {}

_ACT_SET = "natural_log_exp_and_others"


def _pin_act_tables():
    """Make the table-load pass see only one usable activation set so the
    whole kernel shares a single ACT_TABLE_LOAD (Exp/Ln/Square all live in
    natural_log_exp_and_others). List order/length is preserved so the
    emitted act_func_set_id still indexes the real act_info.json."""
    import concourse.bacc as bacc_mod
    from concourse.hw_specs import get_activation_tables

    real = get_activation_tables  # functools.cache'd original

    def patched(arch):
        tabs = real(arch)
        return {
            name: (funcs if name == _ACT_SET else set())
            for name, funcs in tabs.items()
        }

    bacc_mod.get_activation_tables = patched


def _build_nc():
    AluOp = mybir.AluOpType
    ACT = mybir.ActivationFunctionType
    AX = mybir.AxisListType

    _pin_act_tables()
    nc = bacc.Bacc("TRN2", target_bir_lowering=False, debug=False, num_devices=M)

    # recb layout: [iter, partition, chunk, row], chunks 0:4 = x dims,
    # 4:8 = xhat dims -- one contiguous 8KB run per (iter, partition).
    # recf (fp8): [partition, iter, chunk, row], chunk 0 = z dims, chunk 1 =
    # joint|pred logits (rows 0:94); fetched as 4 DMAs with 4KB runs so the
    # descriptor processor is never clogged by small descriptors.
    recb_d = nc.dram_tensor("recb", [NI * P, 8 * RI], BF16, kind="ExternalInput")
    recf_d = nc.dram_tensor("recf", [P, NI * 2 * RI], F8, kind="ExternalInput")
    sel_d = nc.dram_tensor("sel", [P, NG * 4], F32, kind="ExternalInput")
    y_d = nc.dram_tensor("y", [P, NG], F32, kind="ExternalInput")
    seg_d = nc.dram_tensor("segc", [P, 4], BF16, kind="ExternalInput")
    out_d = nc.dram_tensor("out", [1, 2], F32, kind="ExternalOutput")

    with tile.TileContext(nc) as tc, ExitStack() as ctx:
        cpool = ctx.enter_context(tc.tile_pool(name="consts", bufs=1))
        # all record tiles live in SBUF at once (144KB/partition): every
        # DMA is issued up front so no queue ever stalls on a pool buffer
        apool = ctx.enter_context(tc.tile_pool(name="a", bufs=1))
        spool = ctx.enter_context(tc.tile_pool(name="scr", bufs=8))
        fpool = ctx.enter_context(tc.tile_pool(name="fin", bufs=1))
        psp = ctx.enter_context(
            tc.tile_pool(name="ps", bufs=1, space=bass.MemorySpace.PSUM)
        )

        ones1 = cpool.tile([P, 1], BF16)       # moving vector for ssq reduce
        seg = cpool.tile([P, 4], BF16)         # segment matrix, rows 94: zero
        onesf = cpool.tile([P, 1], F32)        # final cross-partition reduce
        sel = cpool.tile([P, NG, 4], F32)      # (jsel, psel0, psel1, psel2)
        yc = cpool.tile([P, NG], F32)          # Y_valid as f32, column layout

        # separate PSUM tiles per combine half so the mid-loop combine of
        # groups 0:32 never creates WAR coupling with PE writes to 32:64
        ps_ssq0 = psp.tile([P, NGH], F32)      # per-row |dx|^2 + |z|^2
        ps_ssq1 = psp.tile([P, NGH], F32)
        ps_sj0 = psp.tile([P, NGH, 4], F32)    # (SJ, S0, S1, S2) per row
        ps_sj1 = psp.tile([P, NGH, 4], F32)
        ps_ssq = [ps_ssq0, ps_ssq1]
        ps_sj = [ps_sj0, ps_sj1]
        ps_f = psp.tile([1, 2], F32)

        # fp8 stream + consts on the scalar queue (few issues -- the scalar
        # engine computes afterwards, so its ring must never back up);
        # recb tiles alternate sync/gpsimd, whose engines only issue DMAs
        tfall = apool.tile([P, NI, 2, RI], F8)
        nc.scalar.dma_start(
            out=tfall[:],
            in_=recf_d[:].rearrange("p (i c r) -> p i c r", c=2, r=RI),
        )
        nc.scalar.dma_start(
            out=sel[:], in_=sel_d[:].rearrange("p (t f) -> p t f", f=4)
        )
        nc.scalar.dma_start(out=yc[:], in_=y_d[:])
        nc.scalar.dma_start(out=seg[:], in_=seg_d[:])
        nc.vector.memset(ones1[:], 1.0)
        nc.vector.memset(onesf[:], 1.0)

        btiles = []
        for i in range(NI):
            ta = apool.tile([P, 8, RI], BF16, tag=f"ta{i}")
            q = nc.sync if i % 2 == 0 else nc.gpsimd
            q.dma_start(
                out=ta[:],
                in_=recb_d[i * P : (i + 1) * P, :].rearrange(
                    "p (c r) -> p c r", r=RI
                ),
            )
            btiles.append(ta)

        def emit_iter(i):
            ta = btiles[i]
            zf = tfall[:, i, 0, :]
            jf = tfall[:, i, 1, :]
            tz2 = spool.tile([P, RI], BF16, tag="tz2")
            te = spool.tile([P, RI], BF16, tag="te")
            # dx = x - xhat, written over the xhat chunks (4:8)
            nc.vector.tensor_tensor(
                out=ta[:, 4:8, :], in0=ta[:, 0:4, :], in1=ta[:, 4:8, :],
                op=AluOp.subtract,
            )
            # square dx in place: ScalarE and VectorE take 2 chunks each;
            # ScalarE also squares z from the fp8 stream
            nc.scalar.activation(out=ta[:, 4:6, :], in_=ta[:, 4:6, :], func=ACT.Square)
            nc.vector.tensor_tensor(
                out=ta[:, 6:8, :], in0=ta[:, 6:8, :], in1=ta[:, 6:8, :],
                op=AluOp.mult,
            )
            nc.scalar.activation(out=tz2[:], in_=zf, func=ACT.Square)
            # exp of the joint|pred logits (rows 94:128 are zero pad)
            nc.scalar.activation(out=te[:], in_=jf, func=ACT.Exp)
            for g in range(GPI):
                gcol = i * GPI + g
                h, gc = divmod(gcol, NGH)
                rs = slice(g * P, (g + 1) * P)
                for c in range(4):
                    nc.tensor.matmul(
                        ps_ssq[h][:, gc : gc + 1], ta[:, 4 + c, rs], ones1[:],
                        start=(c == 0), stop=False,
                    )
                nc.tensor.matmul(
                    ps_ssq[h][:, gc : gc + 1], tz2[:, rs], ones1[:],
                    start=False, stop=True,
                )
                nc.tensor.matmul(
                    ps_sj[h][:, gc, :], te[:, rs], seg[:], start=True, stop=True
                )

        # pointwise combine, emitted per half as soon as its 32 PSUM group
        # columns complete -- half 0 overlaps the stream, and splitting the
        # tail instructions across the program cuts the serialized 16KB
        # instruction-fetch rounds observed after the last tile
        lnsj = fpool.tile([P, NG], F32)
        spp = fpool.tile([P, NG], F32)
        av = fpool.tile([P, NG], F32)
        p3 = fpool.tile([P, NG], F32)
        tv = fpool.tile([P, NG], F32)
        hv = fpool.tile([P, NG], F32)
        ev = fpool.tile([P, NG], F32)
        fb = fpool.tile([P, 2, NG], F32)
        rr = fpool.tile([P, 2], F32)
        osb = fpool.tile([1, 2], F32)
        sjs = fpool.tile([P, NG, 4], F32)

        def emit_half(h):
            s = slice(h * NGH, (h + 1) * NGH)
            nc.vector.tensor_copy(out=sjs[:, s, :], in_=ps_sj[h][:])
            nc.vector.tensor_scalar(
                out=fb[:, 1, s], in0=yc[:, s], scalar1=float(C), scalar2=None,
                op0=AluOp.is_lt,
            )
            nc.vector.tensor_tensor(
                out=spp[:, s], in0=sjs[:, s, 1], in1=sjs[:, s, 2], op=AluOp.mult
            )
            nc.vector.tensor_tensor(
                out=spp[:, s], in0=spp[:, s], in1=sjs[:, s, 3], op=AluOp.mult
            )
            nc.scalar.activation(out=lnsj[:, s], in_=sjs[:, s, 0], func=ACT.Ln)
            nc.scalar.activation(out=spp[:, s], in_=spp[:, s], func=ACT.Ln)
            nc.vector.tensor_tensor(
                out=p3[:, s], in0=sel[:, s, 1], in1=sel[:, s, 2], op=AluOp.add
            )
            nc.vector.tensor_tensor(
                out=p3[:, s], in0=p3[:, s], in1=sel[:, s, 3], op=AluOp.add
            )
            nc.vector.tensor_tensor(
                out=av[:, s], in0=sel[:, s, 0], in1=lnsj[:, s], op=AluOp.subtract
            )
            nc.vector.tensor_tensor(
                out=p3[:, s], in0=p3[:, s], in1=spp[:, s], op=AluOp.subtract
            )
            nc.vector.tensor_tensor(
                out=tv[:, s], in0=av[:, s], in1=p3[:, s], op=AluOp.subtract
            )
            nc.vector.tensor_scalar(
                out=hv[:, s], in0=ps_ssq[h][:], scalar1=-0.5, scalar2=None,
                op0=AluOp.mult,
            )
            nc.vector.tensor_tensor(
                out=hv[:, s], in0=av[:, s], in1=hv[:, s], op=AluOp.add
            )
            nc.scalar.activation(out=ev[:, s], in_=hv[:, s], func=ACT.Exp)
            nc.vector.tensor_tensor(
                out=tv[:, s], in0=tv[:, s], in1=ev[:, s], op=AluOp.mult
            )
            nc.vector.tensor_tensor(
                out=fb[:, 0, s], in0=tv[:, s], in1=fb[:, 1, s], op=AluOp.mult
            )

        for i in range(NI):
            emit_iter(i)
            if i == NI // 2 - 1:
                emit_half(0)
        emit_half(1)

        nc.vector.tensor_reduce(out=rr[:], in_=fb[:], axis=AX.X, op=AluOp.add)
        nc.tensor.matmul(ps_f[:], onesf[:], rr[:], start=True, stop=True)
        nc.vector.tensor_copy(out=osb[:], in_=ps_f[:])
        nc.sync.dma_start(out=out_d[:], in_=osb[:])

    nc.compile()
    return nc


def _get_nc():
    if "nc" not in _NC_CACHE:
        _NC_CACHE["nc"] = _build_nc()
    return _NC_CACHE["nc"]


def _col_layout(arr):
    """[BL, ...] per-core rows -> [P, NG, ...] where row g*128 + p lands at
    [p, g] (the PE-group layout the kernel produces in PSUM)."""
    tail = arr.shape[1:]
    a = arr.reshape(NG, P, *tail)
    a = np.moveaxis(a, 1, 0)
    return np.ascontiguousarray(a)


def _prep_in_maps(inputs):
    X = np.asarray(inputs["X"], dtype=np.float32)
    XH = np.asarray(inputs["X_hat"], dtype=np.float32)
    Z = np.asarray(inputs["Z"], dtype=np.float32)
    JP = np.asarray(inputs["joint_probs"], dtype=np.float32)[:, :C]
    PP = np.asarray(inputs["pred_probs"], dtype=np.float32)      # [3, B, 10]
    y = np.asarray(inputs["Y_valid"])
    vcp = np.asarray(inputs["valid_cp"])

    y_safe = np.where(y < C, y, 0).astype(np.int64)
    v3 = vcp[y_safe].astype(np.int64)                            # [B, 3]
    bidx = np.arange(B)
    sel = np.empty((B, 4), np.float32)
    sel[:, 0] = JP[bidx, y_safe]
    for d in range(D):
        sel[:, d + 1] = PP[d, bidx, v3[:, d]]
    y32 = y.astype(np.float32)

    segc = np.zeros((P, 4), BF16NP)
    segc[0:C, 0] = 1
    for d in range(D):
        segc[C + d * K : C + (d + 1) * K, d + 1] = 1

    Xb = X.astype(BF16NP)
    XHb = XH.astype(BF16NP)
    Zb = Z.astype(F8NP)
    JPb = JP.astype(F8NP)
    PPb = PP.transpose(0, 2, 1).reshape(D * K, B).astype(F8NP)    # [30, B]

    in_maps = []
    for m in range(M):
        s = slice(m * BL, (m + 1) * BL)
        txx = np.concatenate([Xb[s].T, XHb[s].T], axis=0)           # [1024, BL]
        # [c*128+p, i*512+r] -> [i, p, c, r]
        recb = np.ascontiguousarray(
            txx.reshape(8, P, NI, RI).transpose(2, 1, 0, 3)
        )
        recf = np.zeros((P, NI, 2, RI), F8NP)
        recf[:, :, 0, :] = Zb[s].T.reshape(P, NI, RI)
        jpp = np.concatenate([JPb[s].T, PPb[:, s]], axis=0)         # [94, BL]
        recf[:JH, :, 1, :] = jpp.reshape(JH, NI, RI)
        in_maps.append(
            {
                "recb": recb.reshape(NI * P, 8 * RI),
                "recf": recf.reshape(P, NI * 2 * RI),
                "sel": _col_layout(sel[s]).reshape(P, NG * 4),
                "y": _col_layout(y32[s]),
                "segc": segc,
            }
        )
    return in_maps


def _combine(results):
    tot = 0.0
    cnt = 0.0
    for r in results:
        o = np.asarray(r["out"], dtype=np.float64)
        tot += float(o[0, 0])
        cnt += float(o[0, 1])
    loss = abs(tot)
    val = loss / cnt if cnt > 0 else loss
    return np.float32(val)


def run(inputs, trace=False, **kwargs):
    """Build (cached), run on the 8 NeuronCores, return (value, BassKernelResults)."""
    nc = _get_nc()
    in_maps = _prep_in_maps(inputs)
    res = run_bass_kernel_spmd(nc, in_maps, list(range(M)), trace=trace, **kwargs)
    return _combine(res.results), res


def kernel(**inputs):
    val, _ = run(inputs, trace=False)
    return val
